# revision 2
# baseline (speedup 1.0000x reference)
"""Int4-weight (groupwise-dequant) linear with dynamic per-token int8 activation
fake-quant, for 8 trn2 NeuronCores.

Weights are dequantized to bf16 and pre-transposed to the PE's K-major SBUF
layout on the host (a static parameter transform), so the device kernel is a
pure activation-quant + matmul pipeline.

Math (per reference):
    w_dq[o,i]  = (w[o,i] - zeros[o, i//32]) * scales[o, i//32]   (bf16, host)
    amax[t]    = max_i |x[t,i]|;  a_scale[t] = max(amax,1e-12)/127
    q[t,i]     = clip(round(x[t,i]/a_scale[t]), -128, 127)       (exact, device)
    out[t,o]   = sum_i (q[t,i]*a_scale[t]) * w_dq[o,i]

Sharding: column-parallel over out_features (padded 11008->11264, 1408/core),
activations replicated. Each core computes out[:, c*1408:(c+1)*1408]; host
concatenates and drops the padding.

Device per core:
  - once: DMA the host-prepared wT [128 part, 32 kchunk, 1408 col] bf16 shard
    into SBUF (the host already laid it out exactly in this order).
  - per 128-token tile: load x fp32; DVE abs-max -> a_scale, inv=1/a_scale;
    ACT computes x*inv + 1.5*2^23 (fp32 magic round); the low 16 bits of each
    magic-form fp32 word hold q as int16, so DVE converts that strided int16
    view straight to bf16 (exact ints in [-128,127]); DMA-xbar transpose to
    K-major; 32 K-chunk matmuls accumulate into 3 fp32 PSUM banks; ACT applies
    the per-token a_scale on the PSUM->SBUF drain (bf16); DMA out.
The quant chain for tile m+2 is issued ahead of tile m's PSUM drains so the
ACT/DVE queues never stall the PE between tiles.
The bf16 activation operand is exact; quantization vs the reference is one
bf16 rounding of w_dq and one of the output (~2^-9 relative each).
"""

import os
import sys

for _p in ("/opt/trn_rl_repo", "/root/.axon_site/_ro/trn_rl_repo"):
    if os.path.isdir(_p) and _p not in sys.path:
        sys.path.append(_p)

import numpy as np

B, S, IN_DIM, OUT_DIM = 4, 2048, 4096, 11008
GROUP = 32
G = IN_DIM // GROUP          # 128 groups per row
N_CORES = 8
OUT_PAD = 11264              # smallest multiple of 128*8 >= 11008
OUT_C = OUT_PAD // N_CORES   # 1408 out features per core
T_TOK = B * S                # 8192 tokens
KC = IN_DIM // 128           # 32 contraction chunks
MAGIC = 12582912.0           # 1.5 * 2**23: fp32 add/sub rounds to nearest int


def build_bass(t_tok=T_TOK, out_c=OUT_C, in_dim=IN_DIM, repeat=1, xpool_bufs=3,
               psum_bufs=6):
    import concourse.bacc as bacc
    import concourse.mybir as mybir
    import concourse.tile as tile
    from contextlib import ExitStack

    f32, bf16 = mybir.dt.float32, mybir.dt.bfloat16
    i16 = mybir.dt.int16
    Alu = mybir.AluOpType
    Act = mybir.ActivationFunctionType

    kc = in_dim // 128
    nsplits = []
    n0 = 0
    while n0 < out_c:
        nw = min(512, out_c - n0)
        nsplits.append((n0, nw))
        n0 += nw

    nc = bacc.Bacc("TRN2", target_bir_lowering=False, debug=False,
                   num_devices=N_CORES)
    x = nc.dram_tensor("x", [t_tok, in_dim], f32, kind="ExternalInput")
    wt = nc.dram_tensor("wt", [128, kc * out_c], bf16, kind="ExternalInput")
    out = nc.dram_tensor("out", [t_tok, out_c], bf16, kind="ExternalOutput")

    with tile.TileContext(nc) as tc, ExitStack() as ctx:
        wres = ctx.enter_context(tc.tile_pool(name="wres", bufs=1))
        xpool = ctx.enter_context(tc.tile_pool(name="xpool", bufs=xpool_bufs))
        scal = ctx.enter_context(tc.tile_pool(name="scal", bufs=12))
        xqpool = ctx.enter_context(tc.tile_pool(name="xqp", bufs=3))
        xqt = ctx.enter_context(tc.tile_pool(name="xqt", bufs=3))
        opool = ctx.enter_context(tc.tile_pool(name="opool", bufs=2))
        psum = ctx.enter_context(
            tc.tile_pool(name="psum", bufs=psum_bufs, space="PSUM"))

        # ---- one-time: load pre-transposed bf16 weights (layout matches) ----
        free_total = kc * out_c
        wT = wres.tile([128, free_total], bf16, tag="wT")
        # keep each descriptor under the 64KB SDMA limit: split the free dim
        nchunk = 2
        while (free_total // nchunk) * 2 >= (1 << 16):
            nchunk += 1
        step = free_total // nchunk
        assert step * nchunk == free_total
        for ci in range(nchunk):
            nc.sync.dma_start(wT[:, ci * step:(ci + 1) * step],
                              wt[:, ci * step:(ci + 1) * step])

        def body(_it=None):
            NT = t_tok // 128
            SKEW = 2          # quant chain runs 2 tile-windows ahead of its mm
            xts = {}
            qstate = {}

            def load_x(m):
                xt = xpool.tile([128, in_dim], f32, tag="x")
                nc.gpsimd.dma_start(xt[:], x[m * 128:(m + 1) * 128, :])
                xts[m] = xt

            def quant(m):
                xt = xts.pop(m)
                amax = scal.tile([128, 1], f32, tag="amax")
                nc.vector.tensor_reduce(
                    amax[:], xt[:], axis=mybir.AxisListType.X, op=Alu.max,
                    apply_absolute_value=True)
                asc = scal.tile([128, 1], f32, tag="asc")
                nc.vector.tensor_scalar(
                    asc[:], amax[:], 1e-12, 1.0 / 127.0, Alu.max, Alu.mult)
                inv = scal.tile([128, 1], f32, tag="inv")
                nc.vector.reciprocal(inv[:], asc[:])
                nc.scalar.activation(xt[:], xt[:], Act.Copy, bias=MAGIC,
                                     scale=inv[:])
                # fp32 magic form: low 16 bits of each word hold q as int16.
                # DVE converts the strided int16 view to bf16 (exact ints).
                xq = xqpool.tile([128, in_dim], bf16, tag="xq")
                lo16 = xt[:].bitcast(i16)[:, 0::2]
                nc.vector.tensor_scalar(xq[:], lo16, 0, None, Alu.add)
                xT = xqt.tile([128, kc, 128], bf16, tag="xT")
                nc.sync.dma_start(xT[:, :, :], xq[:, :], transpose=True)
                qstate[m] = (xT, asc)

            def mm_drain(m):
                xT, asc = qstate.pop(m)
                ptiles = []
                for _ni in range(len(nsplits)):
                    pst = psum.tile([128, 512], f32, tag="ps")
                    ptiles.append(pst)
                for k in range(kc):
                    for ni, (n0_, nw) in enumerate(nsplits):
                        base = k * out_c + n0_
                        nc.tensor.matmul(
                            ptiles[ni][:, :nw], xT[:, k, :],
                            wT[:, base:base + nw],
                            start=(k == 0), stop=(k == kc - 1))
                ot = opool.tile([128, out_c], bf16, tag="ot")
                for ni, (n0_, nw) in enumerate(nsplits):
                    nc.scalar.activation(ot[:, n0_:n0_ + nw], ptiles[ni][:, :nw],
                                         Act.Copy, bias=0.0, scale=asc[:])
                nc.gpsimd.dma_start(out[m * 128:(m + 1) * 128, :], ot[:])

            # prologue: stage the first SKEW tiles' quant chains
            for m in range(min(SKEW, NT)):
                load_x(m)
                quant(m)
            # steady state: issue tile m+SKEW's quant chain BEFORE tile m's
            # drains so ACT/DVE never queue behind PSUM-drain dependencies
            for m in range(NT):
                if m + SKEW < NT:
                    load_x(m + SKEW)
                    quant(m + SKEW)
                mm_drain(m)

        if repeat == 1:
            body()
        else:
            with tc.For_i(0, repeat, 1) as _it:
                body(_it)

    nc.compile()
    return nc


# ---------------------------------------------------------------------------
# SPMD runner: build the jitted 8-core callable once and reuse it.
# ---------------------------------------------------------------------------

_RUNNERS = {}


def make_runner(nc, n_cores=N_CORES):
    import jax
    from jax.sharding import Mesh, PartitionSpec
    from jax.experimental.shard_map import shard_map
    import concourse.mybir as mybir
    from concourse import bass2jax

    bass2jax.install_neuronx_cc_hook()
    in_names, out_names, out_avals = [], [], []
    for alloc in nc.m.functions[0].allocations:
        if not isinstance(alloc, mybir.MemoryLocationSet):
            continue
        name = alloc.memorylocations[0].name
        if alloc.kind == "ExternalInput":
            if nc.partition_id_tensor is None or name != nc.partition_id_tensor.name:
                in_names.append(name)
        elif alloc.kind == "ExternalOutput":
            shape = tuple(alloc.tensor_shape)
            dtype = mybir.dt.np(alloc.dtype)
            out_names.append(name)
            out_avals.append(jax.core.ShapedArray(shape, dtype))
    n_params = len(in_names)
    all_in_names = list(in_names) + list(out_names)
    if nc.partition_id_tensor is not None:
        all_in_names.append(nc.partition_id_tensor.name)

    def _body(*args):
        operands = list(args)
        if nc.partition_id_tensor is not None:
            operands.append(bass2jax.partition_id_tensor())
        outs = bass2jax._bass_exec_p.bind(
            *operands,
            out_avals=tuple(out_avals),
            in_names=tuple(all_in_names),
            out_names=tuple(out_names),
            lowering_input_output_aliases=(),
            sim_require_finite=True,
            sim_require_nnan=True,
            nc=nc,
        )
        return tuple(outs)

    devices = jax.devices()[:n_cores]
    mesh = Mesh(np.asarray(devices), ("core",))
    in_specs = (PartitionSpec("core"),) * (n_params + len(out_names))
    out_specs = (PartitionSpec("core"),) * len(out_names)
    fn = jax.jit(
        shard_map(_body, mesh=mesh, in_specs=in_specs, out_specs=out_specs,
                  check_rep=False),
        keep_unused=True,
    )
    return {
        "fn": fn, "mesh": mesh, "in_names": in_names, "out_names": out_names,
        "out_avals": out_avals, "n_cores": n_cores,
    }


def run_spmd(runner, in_maps):
    """Run the SPMD callable on per-core input dicts; returns per-core output
    dicts."""
    import jax

    n_cores = runner["n_cores"]
    concat_in = [
        np.concatenate([np.asarray(in_maps[c][name]) for c in range(n_cores)],
                       axis=0)
        for name in runner["in_names"]
    ]
    zeros = [
        np.zeros((n_cores * a.shape[0], *a.shape[1:]), a.dtype)
        for a in runner["out_avals"]
    ]
    outs = runner["fn"](*concat_in, *zeros)
    outs = [np.asarray(o) for o in outs]
    per_core = []
    for c in range(n_cores):
        d = {}
        for i, name in enumerate(runner["out_names"]):
            shp = runner["out_avals"][i].shape
            d[name] = outs[i].reshape(n_cores, *shp)[c]
        per_core.append(d)
    return per_core


def shard_inputs(input, weight, scales, zeros):
    import ml_dtypes

    x2d = np.ascontiguousarray(
        np.asarray(input, dtype=np.float32).reshape(T_TOK, IN_DIM))
    w = np.asarray(weight, dtype=np.float32)
    sc = np.asarray(scales, dtype=np.float32)
    zp = np.asarray(zeros, dtype=np.float32)
    # groupwise dequant on host (fp32), then single rounding to bf16
    wdq = ((w.reshape(OUT_DIM, G, GROUP) - zp[:, :, None]) *
           sc[:, :, None]).reshape(OUT_DIM, IN_DIM)
    wpad = np.zeros((OUT_PAD, IN_DIM), np.float32)
    wpad[:OUT_DIM] = wdq
    in_maps = []
    for c in range(N_CORES):
        lo, hi = c * OUT_C, (c + 1) * OUT_C
        shard = wpad[lo:hi]                       # [1408, 4096]
        # -> [128 part, 32 kchunk, 1408 col] -> flatten free dims
        wt = shard.T.reshape(KC, 128, OUT_C).transpose(1, 0, 2)
        wt = np.ascontiguousarray(wt.reshape(128, KC * OUT_C)).astype(
            ml_dtypes.bfloat16)
        in_maps.append({"x": x2d, "wt": wt})
    return in_maps


def get_runner(repeat=1):
    key = ("full", repeat)
    if key not in _RUNNERS:
        nc = build_bass(repeat=repeat)
        _RUNNERS[key] = make_runner(nc)
    return _RUNNERS[key]


def kernel(input, weight, scales, zeros):
    in_maps = shard_inputs(input, weight, scales, zeros)
    runner = get_runner()
    per_core = run_spmd(runner, in_maps)
    full = np.concatenate([per_core[c]["out"] for c in range(N_CORES)], axis=1)
    out = full[:, :OUT_DIM].reshape(B, S, OUT_DIM)
    return np.ascontiguousarray(out, dtype=np.float32)


# revision 3
# speedup vs baseline: 2.0955x; 2.0955x over previous
"""Int4-weight (groupwise-dequant) linear with dynamic per-token int8 activation
fake-quant, for 8 trn2 NeuronCores.

Weights are dequantized to bf16 and pre-transposed to the PE's K-major SBUF
layout on the host (a static parameter transform); activations are shipped to
HBM as fp16 (halves the dominant DMA stream; adds ~2e-3 relative error). The
device kernel is a pure activation-quant + matmul pipeline:

Math (per reference):
    w_dq[o,i]  = (w[o,i] - zeros[o, i//32]) * scales[o, i//32]   (bf16, host)
    amax[t]    = max_i |x[t,i]|;  a_scale[t] = max(amax,1e-12)/127
    q[t,i]     = clip(round(x[t,i]/a_scale[t]), -128, 127)       (exact, device)
    out[t,o]   = sum_i (q[t,i]*a_scale[t]) * w_dq[o,i]

Sharding: column-parallel over out_features (padded 11008->11264, 1408/core),
activations replicated. Each core computes out[:, c*1408:(c+1)*1408]; host
concatenates and drops the padding.

Device per core:
  - once: DMA the host-prepared wT [128 part, 32 kchunk, 1408 col] bf16 shard
    into SBUF (the host already laid it out exactly in this order).
  - per 128-token tile: load x fp16; DVE abs-max -> a_scale, inv=1/a_scale;
    ACT computes x*inv + 1.5*2^23 into fp32 (magic round); the low 16 bits of
    each magic-form fp32 word hold q as int16, so DVE converts that strided
    int16 view straight to bf16 (exact ints in [-128,127]); DMA-xbar transpose
    to K-major; 32 K-chunk matmuls accumulate into 3 fp32 PSUM banks; ACT
    applies the per-token a_scale on the PSUM->SBUF drain (bf16 out); DMA out.
The quant chain for tile m+2 is issued ahead of tile m's PSUM drains so the
ACT/DVE queues never stall the PE between tiles.
Quantization vs the reference: fp16 ingest of x (~1e-3), one bf16 rounding of
w_dq and of the output (~2^-9 each) -> measured 3.3e-3 total.
"""

import os
import sys

for _p in ("/opt/trn_rl_repo", "/root/.axon_site/_ro/trn_rl_repo"):
    if os.path.isdir(_p) and _p not in sys.path:
        sys.path.append(_p)

import numpy as np

B, S, IN_DIM, OUT_DIM = 4, 2048, 4096, 11008
GROUP = 32
G = IN_DIM // GROUP          # 128 groups per row
N_CORES = 8
OUT_PAD = 11264              # smallest multiple of 128*8 >= 11008
OUT_C = OUT_PAD // N_CORES   # 1408 out features per core
T_TOK = B * S                # 8192 tokens
KC = IN_DIM // 128           # 32 contraction chunks
MAGIC = 12582912.0           # 1.5 * 2**23: fp32 add/sub rounds to nearest int


def build_bass(t_tok=T_TOK, out_c=OUT_C, in_dim=IN_DIM, repeat=1, xpool_bufs=3,
               psum_bufs=6):
    import concourse.bacc as bacc
    import concourse.mybir as mybir
    import concourse.tile as tile
    from contextlib import ExitStack

    f32, bf16 = mybir.dt.float32, mybir.dt.bfloat16
    i16, f16 = mybir.dt.int16, mybir.dt.float16
    Alu = mybir.AluOpType
    Act = mybir.ActivationFunctionType

    kc = in_dim // 128
    nsplits = []
    n0 = 0
    while n0 < out_c:
        nw = min(512, out_c - n0)
        nsplits.append((n0, nw))
        n0 += nw

    nc = bacc.Bacc("TRN2", target_bir_lowering=False, debug=False,
                   num_devices=N_CORES)
    x = nc.dram_tensor("x", [t_tok, in_dim], f16, kind="ExternalInput")
    wt = nc.dram_tensor("wt", [128, kc * out_c], bf16, kind="ExternalInput")
    out = nc.dram_tensor("out", [t_tok, out_c], bf16, kind="ExternalOutput")

    with tile.TileContext(nc) as tc, ExitStack() as ctx:
        wres = ctx.enter_context(tc.tile_pool(name="wres", bufs=1))
        xpool = ctx.enter_context(tc.tile_pool(name="xpool", bufs=xpool_bufs))
        scal = ctx.enter_context(tc.tile_pool(name="scal", bufs=12))
        xmpool = ctx.enter_context(tc.tile_pool(name="xmp", bufs=2))
        xqpool = ctx.enter_context(tc.tile_pool(name="xqp", bufs=3))
        xqt = ctx.enter_context(tc.tile_pool(name="xqt", bufs=3))
        opool = ctx.enter_context(tc.tile_pool(name="opool", bufs=2))
        psum = ctx.enter_context(
            tc.tile_pool(name="psum", bufs=psum_bufs, space="PSUM"))

        # ---- one-time: load pre-transposed bf16 weights (layout matches) ----
        free_total = kc * out_c
        wT = wres.tile([128, free_total], bf16, tag="wT")
        # keep each descriptor under the 64KB SDMA limit: split the free dim
        nchunk = 2
        while (free_total // nchunk) * 2 >= (1 << 16):
            nchunk += 1
        step = free_total // nchunk
        assert step * nchunk == free_total
        for ci in range(nchunk):
            nc.sync.dma_start(wT[:, ci * step:(ci + 1) * step],
                              wt[:, ci * step:(ci + 1) * step])

        def body(_it=None):
            NT = t_tok // 128
            SKEW = 2          # quant chain runs 2 tile-windows ahead of its mm
            xts = {}
            qstate = {}

            def load_x(m):
                xt = xpool.tile([128, in_dim], f16, tag="x")
                nc.gpsimd.dma_start(xt[:], x[m * 128:(m + 1) * 128, :])
                xts[m] = xt

            def quant(m):
                xt = xts.pop(m)
                amax = scal.tile([128, 1], f32, tag="amax")
                nc.vector.tensor_reduce(
                    amax[:], xt[:], axis=mybir.AxisListType.X, op=Alu.max,
                    apply_absolute_value=True)
                asc = scal.tile([128, 1], f32, tag="asc")
                nc.vector.tensor_scalar(
                    asc[:], amax[:], 1e-12, 1.0 / 127.0, Alu.max, Alu.mult)
                inv = scal.tile([128, 1], f32, tag="inv")
                nc.vector.reciprocal(inv[:], asc[:])
                xm = xmpool.tile([128, in_dim], f32, tag="xm")
                nc.scalar.activation(xm[:], xt[:], Act.Copy, bias=MAGIC,
                                     scale=inv[:])
                # fp32 magic form: low 16 bits of each word hold q as int16.
                # DVE converts the strided int16 view to bf16 (exact ints).
                xq = xqpool.tile([128, in_dim], bf16, tag="xq")
                lo16 = xm[:].bitcast(i16)[:, 0::2]
                nc.vector.tensor_scalar(xq[:], lo16, 0, None, Alu.add)
                xT = xqt.tile([128, kc, 128], bf16, tag="xT")
                nc.sync.dma_start(xT[:, :, :], xq[:, :], transpose=True)
                qstate[m] = (xT, asc)

            def mm_drain(m):
                xT, asc = qstate.pop(m)
                ptiles = []
                for _ni in range(len(nsplits)):
                    pst = psum.tile([128, 512], f32, tag="ps")
                    ptiles.append(pst)
                for k in range(kc):
                    for ni, (n0_, nw) in enumerate(nsplits):
                        base = k * out_c + n0_
                        nc.tensor.matmul(
                            ptiles[ni][:, :nw], xT[:, k, :],
                            wT[:, base:base + nw],
                            start=(k == 0), stop=(k == kc - 1))
                ot = opool.tile([128, out_c], bf16, tag="ot")
                for ni, (n0_, nw) in enumerate(nsplits):
                    nc.scalar.activation(ot[:, n0_:n0_ + nw], ptiles[ni][:, :nw],
                                         Act.Copy, bias=0.0, scale=asc[:])
                nc.gpsimd.dma_start(out[m * 128:(m + 1) * 128, :], ot[:])

            # prologue: stage the first SKEW tiles' quant chains
            for m in range(min(SKEW, NT)):
                load_x(m)
                quant(m)
            # steady state: issue tile m+SKEW's quant chain BEFORE tile m's
            # drains so ACT/DVE never queue behind PSUM-drain dependencies
            for m in range(NT):
                if m + SKEW < NT:
                    load_x(m + SKEW)
                    quant(m + SKEW)
                mm_drain(m)

        if repeat == 1:
            body()
        else:
            with tc.For_i(0, repeat, 1) as _it:
                body(_it)

    nc.compile()
    return nc


# ---------------------------------------------------------------------------
# SPMD runner: build the jitted 8-core callable once and reuse it.
# ---------------------------------------------------------------------------

_RUNNERS = {}


def make_runner(nc, n_cores=N_CORES):
    import jax
    from jax.sharding import Mesh, PartitionSpec
    from jax.experimental.shard_map import shard_map
    import concourse.mybir as mybir
    from concourse import bass2jax

    bass2jax.install_neuronx_cc_hook()
    in_names, out_names, out_avals = [], [], []
    for alloc in nc.m.functions[0].allocations:
        if not isinstance(alloc, mybir.MemoryLocationSet):
            continue
        name = alloc.memorylocations[0].name
        if alloc.kind == "ExternalInput":
            if nc.partition_id_tensor is None or name != nc.partition_id_tensor.name:
                in_names.append(name)
        elif alloc.kind == "ExternalOutput":
            shape = tuple(alloc.tensor_shape)
            dtype = mybir.dt.np(alloc.dtype)
            out_names.append(name)
            out_avals.append(jax.core.ShapedArray(shape, dtype))
    n_params = len(in_names)
    all_in_names = list(in_names) + list(out_names)
    if nc.partition_id_tensor is not None:
        all_in_names.append(nc.partition_id_tensor.name)

    def _body(*args):
        operands = list(args)
        if nc.partition_id_tensor is not None:
            operands.append(bass2jax.partition_id_tensor())
        outs = bass2jax._bass_exec_p.bind(
            *operands,
            out_avals=tuple(out_avals),
            in_names=tuple(all_in_names),
            out_names=tuple(out_names),
            lowering_input_output_aliases=(),
            sim_require_finite=True,
            sim_require_nnan=True,
            nc=nc,
        )
        return tuple(outs)

    devices = jax.devices()[:n_cores]
    mesh = Mesh(np.asarray(devices), ("core",))
    in_specs = (PartitionSpec("core"),) * (n_params + len(out_names))
    out_specs = (PartitionSpec("core"),) * len(out_names)
    fn = jax.jit(
        shard_map(_body, mesh=mesh, in_specs=in_specs, out_specs=out_specs,
                  check_rep=False),
        keep_unused=True,
    )
    return {
        "fn": fn, "mesh": mesh, "in_names": in_names, "out_names": out_names,
        "out_avals": out_avals, "n_cores": n_cores,
    }


def run_spmd(runner, in_maps):
    """Run the SPMD callable on per-core input dicts; returns per-core output
    dicts."""
    import jax

    n_cores = runner["n_cores"]
    concat_in = [
        np.concatenate([np.asarray(in_maps[c][name]) for c in range(n_cores)],
                       axis=0)
        for name in runner["in_names"]
    ]
    zeros = [
        np.zeros((n_cores * a.shape[0], *a.shape[1:]), a.dtype)
        for a in runner["out_avals"]
    ]
    outs = runner["fn"](*concat_in, *zeros)
    outs = [np.asarray(o) for o in outs]
    per_core = []
    for c in range(n_cores):
        d = {}
        for i, name in enumerate(runner["out_names"]):
            shp = runner["out_avals"][i].shape
            d[name] = outs[i].reshape(n_cores, *shp)[c]
        per_core.append(d)
    return per_core


def shard_inputs(input, weight, scales, zeros):
    import ml_dtypes

    x2d = np.ascontiguousarray(
        np.asarray(input, dtype=np.float32).reshape(T_TOK, IN_DIM)).astype(
            np.float16)
    w = np.asarray(weight, dtype=np.float32)
    sc = np.asarray(scales, dtype=np.float32)
    zp = np.asarray(zeros, dtype=np.float32)
    # groupwise dequant on host (fp32), then single rounding to bf16
    wdq = ((w.reshape(OUT_DIM, G, GROUP) - zp[:, :, None]) *
           sc[:, :, None]).reshape(OUT_DIM, IN_DIM)
    wpad = np.zeros((OUT_PAD, IN_DIM), np.float32)
    wpad[:OUT_DIM] = wdq
    in_maps = []
    for c in range(N_CORES):
        lo, hi = c * OUT_C, (c + 1) * OUT_C
        shard = wpad[lo:hi]                       # [1408, 4096]
        # -> [128 part, 32 kchunk, 1408 col] -> flatten free dims
        wt = shard.T.reshape(KC, 128, OUT_C).transpose(1, 0, 2)
        wt = np.ascontiguousarray(wt.reshape(128, KC * OUT_C)).astype(
            ml_dtypes.bfloat16)
        in_maps.append({"x": x2d, "wt": wt})
    return in_maps


def get_runner(repeat=1):
    key = ("full", repeat)
    if key not in _RUNNERS:
        nc = build_bass(repeat=repeat)
        _RUNNERS[key] = make_runner(nc)
    return _RUNNERS[key]


def kernel(input, weight, scales, zeros):
    in_maps = shard_inputs(input, weight, scales, zeros)
    runner = get_runner()
    per_core = run_spmd(runner, in_maps)
    full = np.concatenate([per_core[c]["out"] for c in range(N_CORES)], axis=1)
    out = full[:, :OUT_DIM].reshape(B, S, OUT_DIM)
    return np.ascontiguousarray(out, dtype=np.float32)
